# revision 6
# baseline (speedup 1.0000x reference)
"""Trainium2 Bass kernel for nn_CodebookSingleW (vq_codebook).

    W = codebook[indices].reshape(4096, 4096)
    h = c19(x @ W + b1);  out = h @ W.T + b2

Strategy (8 NeuronCores, data-parallel over batch):
  - Each core handles 1024 rows of x. All weight-side tensors replicated.
  - fp8 DoubleRow matmuls: one DR instruction contracts TWO 128-chunks
    (lhsT [128,2,M], rhs [128,2,N]) at 0.5 PE cycles per output column —
    2x the bf16 MAC rate.
  - Precision via a hi/lo e4m3 split on both operands, dropping the lo*lo
    term:  x@W ~= x_hi@W_hi + x_hi@W_lo + x_lo@W_hi.
    W = codebook[idx] quantizes through the 256-entry table: W_hi uses
    q8(cb*S), W_lo the residual q8(cb*S - W_hi) — both host-gathered as
    fp8 bytes (same DMA bytes as a bf16 W). x (and the on-device h)
    split as v_hi = q8(v*16), v_lo = q8(v*16 - v_hi); all terms share one
    scale family so the 3 DR matmuls accumulate into a single psum chain.
    End-to-end rel err ~1.3e-3 (vs 2.6e-2 for direct fp8).
  - Per phase: 3 DR per k-pair instead of 2 bf16 per pair -> 0.75x bf16
    cycles; both phases PE-bound at ~655us/core vs 873us bf16 roofline.
  - phase 1 evict: tanh on ACT (scale/bias per partition), C19 mix on DVE,
    h*16 split to fp8 hi (ACT Identity cast) + lo (DVE subtract) -> SBUF.
  - phase 2: same DR scheme against host-transposed W^T tiles; + b2 on ACT
    evict, DMA outT per core, host reassembles [8192, 4096] f32.
"""

import sys

sys.path.insert(0, "/opt/trn_rl_repo")

import ml_dtypes
import numpy as np

IN_DIM = 4096
H = 4096
K = 256
B = 8192
NCORES = 8
BL = B // NCORES          # 1024 batch rows per core
P = 128
KT = IN_DIM // P          # 32 contraction tiles (phase 1)
MT = H // P               # 32 output-row tiles
NH = BL // 512            # 2 psum halves of the per-core batch
NKP = KT // 2             # 16 k-pairs per chain

E4 = ml_dtypes.float8_e4m3
BF16 = ml_dtypes.bfloat16
SX = 16.0                 # moving-operand scale (x and h)


def _q8(a):
    return np.asarray(a, np.float32).astype(E4).astype(np.float32)


# ---------------------------------------------------------------------------
# Bass program
# ---------------------------------------------------------------------------

def _build_program(repeat=1):
    import concourse.bacc as bacc
    import concourse.mybir as mybir
    import concourse.tile as tile

    AF = mybir.ActivationFunctionType
    ALU = mybir.AluOpType
    DR = mybir.MatmulPerfMode.DoubleRow
    dt = mybir.dt

    nc = bacc.Bacc("TRN2", target_bir_lowering=False, debug=False,
                   num_devices=NCORES)

    # host-tiled fp8 weights, hi/lo interleaved per output tile:
    #   whl[mt, p, t, kt, c] = W_t[kt*128+p, mt*128+c]  (t: 0=hi, 1=lo)
    #   wtl[jt, p, t, mt, c] = W_t[jt*128+c, mt*128+p]  (the W^T layout)
    whl = nc.dram_tensor("whl", [MT, P, 2, KT, P], dt.float8e4,
                         kind="ExternalInput")
    wtl = nc.dram_tensor("wtl", [KT, P, 2, MT, P], dt.float8e4,
                         kind="ExternalInput")
    xh = nc.dram_tensor("xh", [P, KT, BL], dt.float8e4, kind="ExternalInput")
    xl = nc.dram_tensor("xl", [P, KT, BL], dt.float8e4, kind="ExternalInput")
    cpar = nc.dram_tensor("cpar", [P, 7, MT], dt.float32, kind="ExternalInput")
    outt = nc.dram_tensor("outt", [IN_DIM, BL], dt.float32,
                          kind="ExternalOutput")

    with tile.TileContext(nc) as tc:
        with (
            tc.tile_pool(name="resid", bufs=1) as resid,
            tc.tile_pool(name="wp", bufs=4) as wp,
            tc.tile_pool(name="evict", bufs=3) as evict,
            tc.tile_pool(name="psum", bufs=6, space="PSUM") as psum,
        ):
            # DMA order matters: the first pair's W tiles (PE critical path)
            # must land before the 8 MB x bulk load monopolizes the queues.
            cp_sb = resid.tile([P, 7, MT], dt.float32)
            nc.sync.dma_start(cp_sb[:], cpar.ap())
            pre_w = []
            for mt in (0, 1):
                w_t = wp.tile([P, 2, KT, P], dt.float8e4, tag="w",
                              name=f"w_pre{mt}")
                nc.sync.dma_start(w_t[:], whl.ap()[mt])
                pre_w.append(w_t)
            xh_sb = resid.tile([P, KT, BL], dt.float8e4)
            xl_sb = resid.tile([P, KT, BL], dt.float8e4)
            for kt in range(KT):
                nc.sync.dma_start(xh_sb[:, kt], xh.ap()[:, kt])
                nc.sync.dma_start(xl_sb[:, kt], xl.ap()[:, kt])
            hh_sb = resid.tile([P, MT, BL], dt.float8e4)
            hl_sb = resid.tile([P, MT, BL], dt.float8e4)

            # PE p-state warmup on scratch data during the DMA lead-in.
            # Rotate psum tiles: back-to-back reuse of one tile serializes on
            # a ~1us semaphore round-trip per matmul.
            warm = resid.tile([P, 2, 512], dt.float8e4)
            nc.vector.memset(warm[:], 0.0)
            wpss = [psum.tile([P, 512], dt.float32, tag="ps",
                              name=f"warm{i}") for i in range(4)]
            for i in range(16):
                nc.tensor.matmul(wpss[i % 4][:], warm[:, :, :P], warm[:],
                                 start=True, stop=True, perf_mode=DR)

            def col(j, t):  # [P, 1] per-partition param column
                return cp_sb[:, j, t : t + 1]

            # Output-row tiles in PAIRS, kp-major matmul order: 4 psum chains
            # consume each x/h k-chunk repeatedly so the PE keeps pace with
            # the streaming DMA at kernel start.
            for _rep in range(repeat):
                # ---- phase 1: hT = c19(W^T x^T + b1), split to fp8 ----
                for mp in range(MT // 2):
                    mts = (2 * mp, 2 * mp + 1)
                    if mp == 0 and _rep == 0:
                        w_ts = pre_w
                    else:
                        w_ts = []
                        for mt in mts:
                            w_t = wp.tile([P, 2, KT, P], dt.float8e4, tag="w")
                            nc.sync.dma_start(w_t[:], whl.ap()[mt])
                            w_ts.append(w_t)
                    pss = [[psum.tile([P, 512], dt.float32, tag="ps",
                                      name=f"ps_{mp}_{d}_{nh}")
                            for nh in range(NH)] for d in range(2)]
                    for kp in range(NKP):
                        ks = slice(2 * kp, 2 * kp + 2)
                        for d in range(2):
                            w_hi, w_lo = w_ts[d][:, 0, ks], w_ts[d][:, 1, ks]
                            for nh in range(NH):
                                xhs = xh_sb[:, ks, nh * 512 : (nh + 1) * 512]
                                xls = xl_sb[:, ks, nh * 512 : (nh + 1) * 512]
                                ps = pss[d][nh][:]
                                nc.tensor.matmul(ps, w_hi, xhs, perf_mode=DR,
                                                 start=(kp == 0), stop=False)
                                nc.tensor.matmul(ps, w_hi, xls, perf_mode=DR,
                                                 start=False, stop=False)
                                nc.tensor.matmul(ps, w_lo, xhs, perf_mode=DR,
                                                 start=False,
                                                 stop=(kp == NKP - 1))
                    # c19*16: lin16 + 16(1-rho)c*tanh((s+b1)/c), s=psum/2^15
                    for d, mt in enumerate(mts):
                        for nh in range(NH):
                            ps = pss[d][nh]
                            cs = slice(nh * 512, (nh + 1) * 512)
                            tanh_t = evict.tile([P, 512], dt.float32,
                                                tag="tanh")
                            nc.scalar.activation(tanh_t[:], ps[:], AF.Tanh,
                                                 bias=col(1, mt),
                                                 scale=col(0, mt))
                            lin_t = evict.tile([P, 512], dt.float32,
                                               tag="lin")
                            nc.vector.tensor_scalar(lin_t[:], ps[:],
                                                    col(2, mt), col(3, mt),
                                                    ALU.mult, ALU.add)
                            h16 = evict.tile([P, 512], dt.float32, tag="h16")
                            nc.vector.scalar_tensor_tensor(
                                h16[:], tanh_t[:], col(4, mt), lin_t[:],
                                ALU.mult, ALU.add)
                            nc.scalar.activation(hh_sb[:, mt, cs], h16[:],
                                                 AF.Identity)
                            nc.vector.scalar_tensor_tensor(
                                hl_sb[:, mt, cs], h16[:], 1.0,
                                hh_sb[:, mt, cs], ALU.mult, ALU.subtract)

                # ---- phase 2: outT = (W hT)/2^15 + b2 ----
                for jp in range(KT // 2):
                    jts = (2 * jp, 2 * jp + 1)
                    w_ts = []
                    for jt in jts:
                        w_t = wp.tile([P, 2, MT, P], dt.float8e4, tag="w")
                        nc.sync.dma_start(w_t[:], wtl.ap()[jt])
                        w_ts.append(w_t)
                    pss = [[psum.tile([P, 512], dt.float32, tag="ps",
                                      name=f"ps2_{jp}_{d}_{nh}")
                            for nh in range(NH)] for d in range(2)]
                    for kp in range(NKP):
                        ks = slice(2 * kp, 2 * kp + 2)
                        for d in range(2):
                            w_hi, w_lo = w_ts[d][:, 0, ks], w_ts[d][:, 1, ks]
                            for nh in range(NH):
                                hhs = hh_sb[:, ks, nh * 512 : (nh + 1) * 512]
                                hls = hl_sb[:, ks, nh * 512 : (nh + 1) * 512]
                                ps = pss[d][nh][:]
                                nc.tensor.matmul(ps, w_hi, hhs, perf_mode=DR,
                                                 start=(kp == 0), stop=False)
                                # h_lo correction on half the k-pairs:
                                # err 1.3e-2 vs 1.2e-3 full, 17% fewer
                                # PE cycles (see numpy study).
                                if kp % 2 == 0:
                                    nc.tensor.matmul(ps, w_hi, hls,
                                                     perf_mode=DR,
                                                     start=False, stop=False)
                                nc.tensor.matmul(ps, w_lo, hhs, perf_mode=DR,
                                                 start=False,
                                                 stop=(kp == NKP - 1))
                    for d, jt in enumerate(jts):
                        for nh in range(NH):
                            out_t = evict.tile([P, 512], dt.float32,
                                               tag="out")
                            nc.scalar.activation(out_t[:], pss[d][nh][:],
                                                 AF.Identity,
                                                 bias=col(5, jt),
                                                 scale=col(6, jt))
                            nc.sync.dma_start(
                                outt.ap()[jt * P : (jt + 1) * P,
                                          nh * 512 : (nh + 1) * 512],
                                out_t[:],
                            )

    nc.compile()
    return nc


# ---------------------------------------------------------------------------
# kernel entry point
# ---------------------------------------------------------------------------

def prepare(x, codebook, indices, b1, b2, c19_c, c19_rho):
    """Host-side layout prep + program build. Returns (nc, in_maps)."""
    x = np.asarray(x, dtype=np.float32)
    codebook = np.asarray(codebook, dtype=np.float32)
    b1 = np.asarray(b1, dtype=np.float32)
    b2 = np.asarray(b2, dtype=np.float32)
    c19_c = np.asarray(c19_c, dtype=np.float32)
    c19_rho = np.asarray(c19_rho, dtype=np.float32)
    idx = np.asarray(indices).reshape(IN_DIM, H).astype(np.int64)

    # -- codebook hi/lo split on the e4m3 grid (S = power of 2) --
    cb_max = np.abs(codebook).max()
    S = float(2.0 ** np.floor(np.log2(216.0 / max(cb_max, 1e-30))))
    cb_hi = _q8(codebook * S)
    cb_lo = _q8(codebook * S - cb_hi)
    assert np.abs(cb_hi).max() <= 448 and np.isfinite(cb_hi).all()
    cb_hi8 = cb_hi.astype(E4).view(np.uint8)
    cb_lo8 = cb_lo.astype(E4).view(np.uint8)

    sx = SX
    assert np.abs(x).max() * sx < 224.0, "x overflows e4m3 at SX"

    # -- weight layouts (fp8 bytes, hi/lo stacked) --
    def tile_w(cb8, ix):  # [IN, H] bytes -> [MT, P, KT, P]
        w = cb8[ix]
        return w.reshape(KT, P, MT, P).transpose(2, 1, 0, 3)

    whl = np.ascontiguousarray(
        np.stack([tile_w(cb_hi8, idx), tile_w(cb_lo8, idx)], axis=2)
    ).view(E4)                                            # [MT, P, 2, KT, P]
    idxT = np.ascontiguousarray(idx.T)
    wtl = np.ascontiguousarray(
        np.stack([tile_w(cb_hi8, idxT), tile_w(cb_lo8, idxT)], axis=2)
    ).view(E4)                                            # [KT, P, 2, MT, P]

    # -- C19 per-partition params (folded matmul/h scales) --
    c = np.exp(c19_c)
    invc = np.exp(-c19_c)
    rho = 1.0 / (1.0 + np.exp(-c19_rho))
    inv_ps = 1.0 / (S * sx)                               # psum -> xW
    cols = [invc * inv_ps, b1 * invc, rho * sx * inv_ps, sx * rho * b1,
            sx * (1.0 - rho) * c, b2, np.full(H, inv_ps, dtype=np.float32)]
    cpar = np.stack([np.float32(v).reshape(MT, P).T for v in cols], axis=1)
    cpar = np.ascontiguousarray(cpar.astype(np.float32))  # [P, 7, MT]

    # -- per-core x split --
    xs = (x * sx).astype(E4)
    xr = (np.float32(x * sx) - xs.astype(np.float32)).astype(E4)
    in_maps = []
    for cid in range(NCORES):
        def tile_x(a):  # [BL, IN] fp8 -> [P, KT, BL]
            ac = a[cid * BL : (cid + 1) * BL]
            return np.ascontiguousarray(
                ac.T.reshape(KT, P, BL).transpose(1, 0, 2))
        in_maps.append({
            "whl": whl,
            "wtl": wtl,
            "xh": tile_x(xs),
            "xl": tile_x(xr),
            "cpar": cpar,
        })

    nc = _build_program()
    return nc, in_maps


def kernel(x, codebook, indices, b1, b2, c19_c, c19_rho):
    from concourse.bass_utils import run_bass_kernel_spmd

    nc, in_maps = prepare(x, codebook, indices, b1, b2, c19_c, c19_rho)
    res = run_bass_kernel_spmd(nc, in_maps, core_ids=list(range(NCORES)))
    global LAST_RESULTS
    LAST_RESULTS = res

    out = np.empty((B, IN_DIM), dtype=np.float32)
    for cid in range(NCORES):
        out[cid * BL : (cid + 1) * BL] = res.results[cid]["outt"].T
    return out
